# revision 1
# baseline (speedup 1.0000x reference)
"""Trainium2 Bass kernel for nn_Block (dense transformer block, pre-LN).

Sharding (8 cores, no collectives): core c -> (batch b = c//2, query-group
r = c%2).  Every core computes LN1 + K/V projections for ALL 2048 tokens of
its batch element, and Q/attention/out-proj/FFN only for its 1024 queries:
  r=0 -> tokens [0:512) u [1536:2048)     r=1 -> tokens [512:1536)
This keeps the per-core causal-attention work identical (uniform SPMD
program); the causal masks are host-provided per-core data.

All activations live transposed in SBUF ([feature, token]); the host
pre-transposes x and re-transposes the output.  The attention data path
(h, q, k, v, softmax weights) runs in bf16; the residual stream, out-proj
and FFN run in fp32 with float32r matmuls.
"""

import numpy as np
from contextlib import ExitStack

EMBED = 1024
HEADS = 16
HD = 64
FF = 4096
T = 2048
TH = 1024  # tokens (queries) per core
B = 4
EPS = 1e-5
SCALE = float(EMBED) ** -0.5  # 0.03125
N_CORES = 8
NE = EMBED // 128  # 8 e-tiles
NHP = 8  # head pairs per core (all 16 heads)
ST = (8, 16)  # s-tiles per chunk slot (uniform across cores)

_NC = None


def _q_index(r: int) -> np.ndarray:
    if r == 0:
        return np.concatenate([np.arange(0, 512), np.arange(1536, 2048)])
    return np.arange(512, 1536)


def _q0s(r: int):
    # true start token of each 512-query chunk slot
    return (0, 1536) if r == 0 else (512, 1024)


class _Ctx:
    """Holds the bass module, pools and constants shared across phases."""


def _setup(C):
    import concourse.bass as bass
    import concourse.bacc as bacc
    import concourse.tile as tile
    from concourse import mybir
    from concourse.masks import make_identity

    C.mybir = mybir
    C.f32 = mybir.dt.float32
    C.f32r = mybir.dt.float32r
    C.bf16 = mybir.dt.bfloat16
    C.FT = mybir.ActivationFunctionType
    C.ALU = mybir.AluOpType

    nc = bacc.Bacc("TRN2", target_bir_lowering=False, debug=False,
                   num_devices=N_CORES)
    C.nc = nc
    f32, bf16 = C.f32, C.bf16

    C.d_xkv = nc.dram_tensor("xkv", [EMBED, T], C.f32r, kind="ExternalInput").ap()
    C.d_xq = nc.dram_tensor("xq", [EMBED, TH], C.f32r, kind="ExternalInput").ap()
    C.d_masks = nc.dram_tensor("masks", [16, 128, 512], bf16,
                               kind="ExternalInput").ap()
    C.d_wq = nc.dram_tensor("wqs", [EMBED, EMBED], bf16,
                            kind="ExternalInput").ap()
    C.d_wk = nc.dram_tensor("wks", [EMBED, EMBED], bf16,
                            kind="ExternalInput").ap()
    C.d_wv = nc.dram_tensor("wvs", [EMBED, EMBED], bf16,
                            kind="ExternalInput").ap()
    C.d_wp = nc.dram_tensor("wps", [EMBED, EMBED], C.f32r, kind="ExternalInput").ap()
    C.d_w1 = nc.dram_tensor("w1s", [EMBED, FF], C.f32r, kind="ExternalInput").ap()
    C.d_w2 = nc.dram_tensor("w2s", [FF, EMBED], C.f32r, kind="ExternalInput").ap()
    # vec8 columns: 0:8 g1 | 8:16 beta1 | 16:24 g2 | 24:32 beta2 | 32:40 b_proj
    # | 40:48 bf2 ; bf1v separate [128, 32]
    C.d_vec = nc.dram_tensor("vec8", [128, 48], f32, kind="ExternalInput").ap()
    C.d_bf1 = nc.dram_tensor("bf1v", [128, 32], f32, kind="ExternalInput").ap()
    C.d_out = nc.dram_tensor("out", [EMBED, TH], f32, kind="ExternalOutput").ap()

    C.tile = tile
    C.make_identity = make_identity


def _consts(C, es):
    nc, tc, f32, bf16 = C.nc, C.tc, C.f32, C.bf16
    constp = es.enter_context(tc.tile_pool(name="const", bufs=1))
    C.ps_mm = es.enter_context(tc.tile_pool(name="ps_mm", bufs=4, space="PSUM"))
    dramp = es.enter_context(tc.tile_pool(name="dram", bufs=1, space="DRAM"))
    C.wpool = es.enter_context(tc.tile_pool(name="wtile", bufs=12))
    C.smallp = es.enter_context(tc.tile_pool(name="small", bufs=1))
    C.tmpp = es.enter_context(tc.tile_pool(name="tmp", bufs=2))
    C.stagep = es.enter_context(tc.tile_pool(name="stage", bufs=8))

    identity = constp.tile([128, 128], f32, name="identity")
    C.make_identity(nc, identity[:])
    C.identity_bf = constp.tile([128, 128], bf16, name="identity_bf")
    nc.vector.tensor_copy(C.identity_bf[:], identity[:])
    ones_col_f = constp.tile([128, 1], f32, name="ones_col_f")
    nc.vector.memset(ones_col_f[:], 1.0)
    C.ones_col = constp.tile([128, 1], C.f32r)
    nc.vector.tensor_copy(C.ones_col[:], ones_col_f[:])
    ones_row_f = constp.tile([1, 128], f32, name="ones_row_f")
    nc.vector.memset(ones_row_f[:], 1.0)
    C.ones_row = constp.tile([1, 128], C.f32r)
    nc.vector.tensor_copy(C.ones_row[:], ones_row_f[:])
    C.eps_t = constp.tile([1, 1], f32)
    nc.vector.memset(C.eps_t[:], EPS)
    C.vec8 = constp.tile([128, 48], f32)
    nc.sync.dma_start(out=C.vec8[:], in_=C.d_vec[:])
    C.bf1v = constp.tile([128, 32], f32)
    nc.sync.dma_start(out=C.bf1v[:], in_=C.d_bf1[:])
    C.wpool.tile([128, 128], bf16, tag="wkv", name="wt_pre")
    C.aout_es = ExitStack()
    aoutp = C.aout_es.enter_context(
        tc.tile_pool(name="aout", bufs=1, side="right"))
    C.aout = [aoutp.tile([128, TH], C.f32r, name=f"aout{i}")
              for i in range(NHP)]


def _vcol(C, idx):
    return C.vec8[:, idx:idx + 1]


def _load_staged(C, dram_ap, dst_tiles, Ttot):
    """DMA -> staging tile -> DVE copy, so consumers of dst depend only on
    the DVE semaphore (PE matmuls allow at most 2 sync waits)."""
    nc = C.nc
    for e in range(NE):
        for n in range(Ttot // 512):
            sl = slice(n * 512, (n + 1) * 512)
            st = C.stagep.tile([128, 512], dst_tiles[e].dtype, tag="stage",
                               name="st")
            nc.sync.dma_start(out=st[:],
                              in_=dram_ap[e * 128:(e + 1) * 128, sl])
            nc.scalar.copy(dst_tiles[e][:, sl], st[:])


def _layernorm_t(C, src_tiles, dst_tiles, Ttot, gcol, bcol, pipe=True):
    """LN over the partition (feature) axis of NE x [128, Ttot] tiles.

    pipe=True defers each chunk's apply past the next chunk's stats so the
    PE stats matmuls are not queued behind apply ops on DVE (needs 4 free
    PSUM banks for the broadcast pool; LN2 runs with pipe=False).
    """
    nc, ALU, FT, f32 = C.nc, C.ALU, C.FT, C.f32
    nch = Ttot // 512

    def apply(ps_a, ps_b, sl):
        for e in range(NE):
            t1 = C.tmpp.tile([128, 512], f32, tag="ln_t1", name="t1")
            nc.vector.tensor_mul(t1[:], src_tiles[e][:, sl], ps_a[:])
            nc.vector.tensor_add(t1[:], t1[:], ps_b[:])
            nc.scalar.activation(dst_tiles[e][:, sl], t1[:], FT.Identity,
                                 bias=bcol, scale=gcol)

    with ExitStack() as ln_es:
        ps_st = ln_es.enter_context(
            C.tc.tile_pool(name="ps_st", bufs=1, space="PSUM"))
        ps_bc = ln_es.enter_context(
            C.tc.tile_pool(name="ps_bc", bufs=1, space="PSUM"))
        pend = []
        for n in range(nch):
            sl = slice(n * 512, (n + 1) * 512)
            ps_sum = ps_st.tile([1, 512], f32, tag="sum", name="ps_sum")
            nc.vector.memset(ps_sum[:], 0.0)
            ps_sq = ps_st.tile([1, 512], f32, tag="sq", name="ps_sq")
            nc.vector.memset(ps_sq[:], 0.0)
            for e in range(NE):
                sq = C.tmpp.tile([128, 512], C.f32r, tag="ln_sq", name="sq")
                nc.gpsimd.tensor_mul(sq[:], src_tiles[e][:, sl],
                                     src_tiles[e][:, sl])
                nc.tensor.matmul(ps_sum[:], C.ones_col[:],
                                 src_tiles[e][:, sl],
                                 start=(e == 0), stop=(e == NE - 1))
                nc.tensor.matmul(ps_sq[:], C.ones_col[:], sq[:],
                                 start=(e == 0), stop=(e == NE - 1))
            if pipe and pend:
                apply(*pend.pop(0))
            mu = C.smallp.tile([1, 512], f32, tag="sv0", name="mu")
            nc.vector.tensor_scalar_mul(mu[:], ps_sum[:], 1.0 / EMBED)
            ms = C.smallp.tile([1, 512], C.f32r, tag="sv1", name="ms")
            nc.vector.tensor_scalar_mul(ms[:], ps_sq[:], 1.0 / EMBED)
            t2 = C.smallp.tile([1, 512], C.f32r, tag="sv2", name="t2")
            nc.vector.tensor_mul(t2[:], mu[:], mu[:])
            nc.vector.tensor_sub(ms[:], ms[:], t2[:])          # ms = var
            nc.scalar.activation(ms[:], ms[:], FT.Sqrt, bias=C.eps_t[:])
            with nc.allow_low_precision(reason="rstd consumed by f32r matmul"):
                nc.vector.reciprocal(t2[:], ms[:])             # t2 = rstd
            nc.vector.scalar_tensor_tensor(ms[:], mu[:], -1.0, t2[:],
                                           op0=ALU.mult, op1=ALU.mult)
            # t2 = rstd, ms = -mu*rstd
            ps_a = ps_bc.tile([128, 512], f32, tag="bca", name="ps_a")
            nc.vector.memset(ps_a[:], 0.0)
            nc.tensor.matmul(ps_a[:], C.ones_row[:], t2[:],
                             start=True, stop=True)
            ps_b = ps_bc.tile([128, 512], f32, tag="bcb", name="ps_b")
            nc.vector.memset(ps_b[:], 0.0)
            nc.tensor.matmul(ps_b[:], C.ones_row[:], ms[:],
                             start=True, stop=True)
            pend.append((ps_a, ps_b, sl))
            if not pipe:
                apply(*pend.pop(0))
        while pend:
            apply(*pend.pop(0))


def _proj_pair(C, w_dram, hp, src_tiles, dst, Ttot, copy_eng="act"):
    """dst[128, Ttot] (bf16) = (w[:, hp*128:+128]).T @ src ; bf16 matmuls."""
    nc, f32, bf16 = C.nc, C.f32, C.bf16
    w_t = []
    for e in range(NE):
        wt = C.wpool.tile([128, 128], bf16, tag="wkv", name="wt")
        nc.sync.dma_start(out=wt[:], in_=w_dram[e * 128:(e + 1) * 128,
                                               hp * 128:(hp + 1) * 128])
        w_t.append(wt)
    for n in range(Ttot // 512):
        sl = slice(n * 512, (n + 1) * 512)
        ps = C.ps_mm.tile([128, 512], f32, tag="mm", name="ps")
        for e in range(NE):
            nc.tensor.matmul(ps[:], w_t[e][:], src_tiles[e][:, sl],
                             start=(e == 0), stop=(e == NE - 1))
        if copy_eng == "act":
            nc.scalar.copy(dst[:, sl], ps[:])
        else:
            nc.vector.tensor_copy(dst[:, sl], ps[:])


def _attention_hp(C, hp, kT, vT, qT, mask_t, vnp, expp):
    nc, f32, bf16, FT = C.nc, C.f32, C.bf16, C.FT
    r_ = C.r_

    # V -> natural layout [tokens, 2x64 feats] with ones cols at 64 / 129
    vn = []
    for s in range(16):
        ps_t = C.ps_mm.tile([128, 128], bf16, tag="mm", name="ps_t")
        nc.tensor.transpose(ps_t[:], vT[:, s * 128:(s + 1) * 128],
                            C.identity_bf[:])
        vt = vnp.tile([128, 130], bf16, name="vt")
        nc.vector.tensor_copy(vt[:, 0:64], ps_t[:, 0:64])
        nc.vector.tensor_copy(vt[:, 65:129], ps_t[:, 64:128])
        nc.vector.memset(vt[:, 64:65], 1.0)
        nc.vector.memset(vt[:, 129:130], 1.0)
        vn.append(vt)

    aout = C.aout[hp]
    for slot in range(2):
        n_st = ST[slot]
        qsl = slice(slot * 512, (slot + 1) * 512)
        pso = [C.ps_acc.tile([128, 512], f32, tag="acc", name="pso")
               for _ in range(2)]
        nc.vector.memset(pso[0][:], 0.0)
        nc.vector.memset(pso[1][:], 0.0)
        for si in range(n_st):
            for a in range(2):
                hsl = slice(a * 64, (a + 1) * 64)
                ps_s = C.ps_mm.tile([128, 512], f32, tag="mm", name="ps_s")
                nc.tensor.matmul(ps_s[:], kT[hsl, si * 128:(si + 1) * 128],
                                 qT[hp][hsl, qsl], start=True, stop=True)
                ex = expp.tile([128, 512], bf16, name="ex")
                nc.scalar.activation(ex[:], ps_s[:], FT.Exp, scale=SCALE)
                if (slot == 0) or (si >= 8):
                    exm = expp.tile([128, 512], bf16, tag="exm", name="exm")
                    if a == 0:
                        nc.vector.tensor_mul(exm[:], ex[:], mask_t[si][:])
                    else:
                        nc.gpsimd.tensor_mul(exm[:], ex[:], mask_t[si][:])
                    ex = exm
                lo = 65 * a
                nc.tensor.matmul(pso[a][0:65, :], vn[si][:, lo:lo + 65],
                                 ex[:], start=(si == 0), stop=(si == n_st - 1))
        for a in range(2):
            rd = C.smallp.tile([1, 512], C.f32r, tag="sv0", name="rd")
            dcp = C.smallp.tile([1, 512], f32, tag="sv1", name="dcp")
            nc.vector.tensor_copy(dcp[:], pso[a][64:65, :])
            with nc.allow_low_precision(reason="softmax denom, f32r matmul"):
                nc.vector.reciprocal(rd[:], dcp[:])
            ps_bc = C.ps_acc.tile([128, 512], f32, tag="bcd", name="ps_bc")
            nc.vector.memset(ps_bc[:], 0.0)
            nc.tensor.matmul(ps_bc[0:64, :], C.ones_row[:, 0:64], rd[:],
                             start=True, stop=True)
            bc = C.tmpp.tile([64, 512], f32, tag="bc_sb", name="bc")
            nc.vector.tensor_copy(bc[:], ps_bc[0:64, :])
            nc.vector.tensor_mul(aout[a * 64:(a + 1) * 64, qsl],
                                 pso[a][0:64, :], bc[:])



def _phase_attention(C, es):
    nc, tc, f32, bf16 = C.nc, C.tc, C.f32, C.bf16
    with ExitStack() as ph:
        hTp = ph.enter_context(tc.tile_pool(name="hT", bufs=1))
        qTp = ph.enter_context(tc.tile_pool(name="qT", bufs=1))
        hT = [hTp.tile([128, T], bf16, name=f"hT{i}") for i in range(NE)]
        qT = [qTp.tile([128, TH], bf16, name=f"qT{i}") for i in range(NHP)]

        with ExitStack() as pa:
            xkvp = pa.enter_context(tc.tile_pool(name="xkv", bufs=1))
            xkv_t = [xkvp.tile([128, T], C.f32r, name=f"xkv_t{i}") for i in range(NE)]
            _load_staged(C, C.d_xkv, xkv_t, T)
            _layernorm_t(C, xkv_t, hT, T, _vcol(C, 0), _vcol(C, 8))

        with ExitStack() as pb:
            hqsp = pb.enter_context(tc.tile_pool(name="hq_src", bufs=1))
            hqp = pb.enter_context(tc.tile_pool(name="hq", bufs=1))
            xq_t = [hqsp.tile([128, TH], C.f32r, name=f"xq_t{i}") for i in range(NE)]
            _load_staged(C, C.d_xq, xq_t, TH)
            hq = [hqp.tile([128, TH], bf16, name=f"hq{i}") for i in range(NE)]
            _layernorm_t(C, xq_t, hq, TH, _vcol(C, 0), _vcol(C, 8))
            for hp in range(NHP):
                _proj_pair(C, C.d_wq, hp, hq, qT[hp], TH)

        with ExitStack() as ph2:
            maskp = ph2.enter_context(tc.tile_pool(name="mask", bufs=1))
            kvp = ph2.enter_context(tc.tile_pool(name="kv", bufs=2))
            vnp = ph2.enter_context(tc.tile_pool(name="vn", bufs=20))
            expp = ph2.enter_context(tc.tile_pool(name="exp", bufs=8))
            C.ps_acc = ph2.enter_context(
                tc.tile_pool(name="ps_acc", bufs=2, space="PSUM"))
            mask_t = []
            for j in range(16):
                mt = maskp.tile([128, 512], bf16, name=f"mt{j}")
                nc.sync.dma_start(out=mt[:], in_=C.d_masks[j, :, :])
                mask_t.append(mt)
            for hp in range(NHP):
                kT = kvp.tile([128, T], bf16, tag="kT", name="kT")
                vT = kvp.tile([128, T], bf16, tag="vT", name="vT")
                _proj_pair(C, C.d_wk, hp, hT, kT, T, copy_eng="dve")
                _proj_pair(C, C.d_wv, hp, hT, vT, T, copy_eng="dve")
                _attention_hp(C, hp, kT, vT, qT, mask_t, vnp, expp)


def _phase_ffn(C, es):
    nc, tc, f32, ALU, FT = C.nc, C.tc, C.f32, C.ALU, C.FT
    r_ = C.r_
    with ExitStack() as ph:
        res1p = ph.enter_context(tc.tile_pool(name="res1", bufs=1))
        wslabp = ph.enter_context(tc.tile_pool(name="wslab", bufs=9))
        res1 = [res1p.tile([128, TH], C.f32r, name=f"res1{i}") for i in range(NE)]

        with ExitStack() as pd:
            ainp = pd.enter_context(tc.tile_pool(name="attn_in", bufs=1))
            attn_in = C.aout
            xres = [ainp.tile([128, TH], C.f32r, name=f"xres{i}")
                    for i in range(NE)]
            for _ in range(4):
                ps_f = C.ps_mm.tile([128, 512], f32, tag="mm", name="ps_flush")
                nc.vector.memset(ps_f[:], 0.0)
            _load_staged(C, C.d_xq, xres, TH)
            wp_slab = []
            for k in range(NE):
                ws = wslabp.tile([128, EMBED], C.f32r, tag="wslab", name="ws")
                nc.sync.dma_start(out=ws[:],
                                  in_=C.d_wp[k * 128:(k + 1) * 128, :])
                wp_slab.append(ws)
            for m in range(NE):
                msl = slice(m * 128, (m + 1) * 128)
                for n in range(2):
                    sl = slice(n * 512, (n + 1) * 512)
                    ps = C.ps_mm.tile([128, 512], f32, tag="mm", name="ps")
                    for k in range(NE):
                        nc.tensor.matmul(ps[:], wp_slab[k][:, msl],
                                         attn_in[k][:, sl],
                                         start=(k == 0), stop=(k == NE - 1))
                    nc.vector.scalar_tensor_tensor(
                        res1[m][:, sl], ps[:], _vcol(C, 32 + m), xres[m][:, sl],
                        op0=ALU.add, op1=ALU.add)

        C.aout_es.close()
        h2p = ph.enter_context(tc.tile_pool(name="h2", bufs=1))
        f1p = ph.enter_context(tc.tile_pool(name="f1", bufs=9))
        f2p = ph.enter_context(tc.tile_pool(name="f2", bufs=1))


        h2 = [h2p.tile([128, TH], C.f32r, name=f"h2{i}") for i in range(NE)]
        _layernorm_t(C, res1, h2, TH, _vcol(C, 16), _vcol(C, 24), pipe=False)

        f2sb = [f2p.tile([128, TH], f32, name=f"f2sb{i}") for i in range(NE)]
        for m2 in range(NE):
            nc.vector.memset(f2sb[m2][:], 0.0)

        for fg in range(4):
            fsl = slice(fg * 1024, (fg + 1) * 1024)
            w1_slab = []
            for e in range(NE):
                ws = wslabp.tile([128, 1024], C.f32r, tag="wslab", name="ws")
                nc.sync.dma_start(out=ws[:],
                                  in_=C.d_w1[e * 128:(e + 1) * 128, fsl])
                w1_slab.append(ws)
            f1_t = []
            for fl in range(8):
                f = fg * 8 + fl
                lsl = slice(fl * 128, (fl + 1) * 128)
                f1 = f1p.tile([128, TH], C.f32r, name="f1")
                for n in range(2):
                    sl = slice(n * 512, (n + 1) * 512)
                    ps = C.ps_mm.tile([128, 512], f32, tag="mm", name="ps")
                    for e in range(NE):
                        nc.tensor.matmul(ps[:], w1_slab[e][:, lsl],
                                         h2[e][:, sl],
                                         start=(e == 0), stop=(e == NE - 1))
                    nc.vector.tensor_scalar(f1[:, sl], ps[:],
                                            C.bf1v[:, f:f + 1], 0.0,
                                            op0=ALU.add, op1=ALU.max)
                f1_t.append(f1)
            w2_slab = []
            for fl in range(8):
                f = fg * 8 + fl
                ws = wslabp.tile([128, EMBED], C.f32r, tag="wslab", name="ws")
                nc.sync.dma_start(out=ws[:],
                                  in_=C.d_w2[f * 128:(f + 1) * 128, :])
                w2_slab.append(ws)
            for m2 in range(NE):
                msl = slice(m2 * 128, (m2 + 1) * 128)
                for n in range(2):
                    sl = slice(n * 512, (n + 1) * 512)
                    ps = C.ps_mm.tile([128, 512], f32, tag="mm", name="ps")
                    for fl in range(8):
                        nc.tensor.matmul(ps[:], w2_slab[fl][:, msl],
                                         f1_t[fl][:, sl],
                                         start=(fl == 0), stop=(fl == 7))
                    nc.vector.tensor_add(f2sb[m2][:, sl], f2sb[m2][:, sl],
                                         ps[:])

        for m2 in range(NE):
            for n in range(2):
                sl = slice(n * 512, (n + 1) * 512)
                ot = C.stagep.tile([128, 512], f32, tag="stage", name="ot")
                nc.vector.scalar_tensor_tensor(
                    ot[:], f2sb[m2][:, sl], _vcol(C, 40 + m2),
                    res1[m2][:, sl], op0=ALU.add, op1=ALU.add)
                nc.sync.dma_start(out=C.d_out[m2 * 128:(m2 + 1) * 128, sl],
                                  in_=ot[:])


def _build_nc():
    C = _Ctx()
    _setup(C)
    C.r_ = lambda ap: ap.bitcast(C.f32r)
    with C.tile.TileContext(C.nc) as tc:
        C.tc = tc
        with ExitStack() as es:
            _consts(C, es)
            _phase_attention(C, es)
            _phase_ffn(C, es)
    C.nc.compile()
    return C.nc


def _get_nc():
    global _NC
    if _NC is None:
        _NC = _build_nc()
    return _NC


def _make_in_maps(x, wq, wk, wv, w_proj, b_proj, g1, beta1, g2, beta2,
                  w1, bf1, w2, bf2):
    import ml_dtypes

    def colify(v, ne):  # [ne*128] -> [128, ne]
        return np.ascontiguousarray(np.asarray(v, np.float32).reshape(ne, 128).T)

    vec8 = np.concatenate(
        [colify(g1, 8), colify(beta1, 8), colify(g2, 8), colify(beta2, 8),
         colify(b_proj, 8), colify(bf2, 8)], axis=1)
    bf1v = colify(bf1, 32)
    bf = ml_dtypes.bfloat16
    wq_s = np.ascontiguousarray(
        np.asarray(wq, np.float32).transpose(1, 0, 2).reshape(EMBED, EMBED)
    ).astype(bf)
    wk_s = np.ascontiguousarray(
        np.asarray(wk, np.float32).transpose(1, 0, 2).reshape(EMBED, EMBED)
    ).astype(bf)
    wv_s = np.ascontiguousarray(
        np.asarray(wv, np.float32).transpose(1, 0, 2).reshape(EMBED, EMBED)
    ).astype(bf)
    w_proj = np.ascontiguousarray(np.asarray(w_proj, np.float32))
    w1 = np.ascontiguousarray(np.asarray(w1, np.float32))
    w2 = np.ascontiguousarray(np.asarray(w2, np.float32))

    s_idx = np.arange(128)[None, :, None]
    t_idx = np.arange(512)[None, None, :]
    j_idx = np.arange(8)[:, None, None]

    in_maps = []
    for core in range(N_CORES):
        b, r = core // 2, core % 2
        q0a, q0b = _q0s(r)
        mA = (j_idx * 128 + s_idx) <= (q0a + t_idx)
        mB = ((j_idx + 8) * 128 + s_idx) <= (q0b + t_idx)
        masks = np.concatenate([mA, mB]).astype(bf)
        qi = _q_index(r)
        xb = np.asarray(x[b], np.float32)
        in_maps.append({
            "xkv": np.ascontiguousarray(xb.T),
            "xq": np.ascontiguousarray(xb[qi].T),
            "masks": masks,
            "wqs": wq_s, "wks": wk_s, "wvs": wv_s, "wps": w_proj,
            "w1s": w1, "w2s": w2, "vec8": vec8, "bf1v": bf1v,
        })
    return in_maps


def _assemble(results):
    out = np.empty((B, T, EMBED), dtype=np.float32)
    for core in range(N_CORES):
        b, r = core // 2, core % 2
        out[b, _q_index(r), :] = results[core]["out"].T
    return out


def kernel(**inputs):
    import time
    from concourse.bass_utils import run_bass_kernel_spmd

    inputs = {k: np.asarray(v) for k, v in inputs.items()}
    nc = _get_nc()
    in_maps = _make_in_maps(**inputs)
    last = None
    for attempt in range(3):
        try:
            res = run_bass_kernel_spmd(nc, in_maps,
                                       core_ids=list(range(N_CORES)))
            return _assemble(res.results)
        except Exception as e:  # transient NRT_EXEC_UNIT_UNRECOVERABLE wedges
            last = e
            if "UNRECOVERABLE" not in str(e) and "UNAVAILABLE" not in str(e):
                raise
            time.sleep(5)
    raise last

